# revision 15
# baseline (speedup 1.0000x reference)
"""Trainium2 Bass kernel for nn_FeatureRefiner (gnn_message_passing).

Computes, for full inputs:
    padded_o = zeros[60000,1024].at[o_ava_idx].set(msg_from_o)
    padded_s = zeros[60000,1024].at[s_ava_idx].set(msg_from_s)
    summ   = 0.5*(padded_o + padded_s)
    out    = summ + relu(summ @ w1_W.T + w1_b) + relu(tar_feat @ w2_W.T + w2_b)

Strategy: data-parallel over rows across 8 NeuronCores. Message rows need
both matmuls; rows with no incoming message (summ == 0) only need the w2
matmul plus a relu(w1_b) broadcast, so each core gets 1/8 of the message
rows and 1/8 of the tail rows to balance FLOPs. Matmul operands are
host-pre-transposed bf16 (summ^T is formed on-device as 0.5*(o^T + s^T)
with the 0.5 folded into w1t); the fp32 summ addend uses natural-layout
fp32 message tiles so it stays exact. Biases ride the PE as a ones⊗bias
matmul appended to each accumulation group; ACT applies ReLU on PSUM
eviction.
"""

import sys

sys.path.insert(0, "/opt/trn_rl_repo")

import numpy as np
import ml_dtypes

FEAT = 1024
P = 128
KC = FEAT // P  # 8 contraction chunks
NH = 512  # matmul moving free dim (one PSUM bank of fp32)
MG = 4  # blocks per transposed-operand load group
N_CORES = 8

BF16 = ml_dtypes.bfloat16

TRACE = False
TRACE_KWARGS = {}
LAST_RESULT = None

_BUILD_CACHE = {}


def _build(nA_blk, nB_blk):
    """Build the single-core Tile program (same program runs SPMD on all cores).

    nA_blk: number of 128-row blocks of message rows (both matmuls)
    nB_blk: number of 128-row blocks of tail rows (w2 matmul only)
    """
    import concourse.bass as bass  # noqa: F401
    import concourse.bacc as bacc
    import concourse.mybir as mybir
    from concourse import tile

    f32 = mybir.dt.float32
    bf16 = mybir.dt.bfloat16
    add = mybir.AluOpType.add
    mult = mybir.AluOpType.mult
    relu = mybir.ActivationFunctionType.Relu

    nA = nA_blk * P
    nB = nB_blk * P

    nc = bacc.Bacc("TRN2", target_bir_lowering=False, debug=False)

    o_h = nc.declare_dram_parameter("o", [nA, FEAT], bf16, isOutput=False)
    s_h = nc.declare_dram_parameter("s", [nA, FEAT], bf16, isOutput=False)
    oT_h = nc.declare_dram_parameter("oT", [FEAT, nA], bf16, isOutput=False)
    sT_h = nc.declare_dram_parameter("sT", [FEAT, nA], bf16, isOutput=False)
    tT_h = nc.declare_dram_parameter("tT", [FEAT, nA + nB], bf16, isOutput=False)
    w1_h = nc.declare_dram_parameter("w1t", [FEAT, FEAT], bf16, isOutput=False)
    w2_h = nc.declare_dram_parameter("w2t", [FEAT, FEAT], bf16, isOutput=False)
    b1_h = nc.declare_dram_parameter("b1bc", [P, FEAT], f32, isOutput=False)
    b2_h = nc.declare_dram_parameter("b2bc", [P, FEAT], f32, isOutput=False)
    out_h = nc.declare_dram_parameter("out", [nA + nB, FEAT], f32, isOutput=True)

    with tile.TileContext(nc) as tc:
        with (
            tc.tile_pool(name="const", bufs=1) as cpool,
            tc.tile_pool(name="grp", bufs=2) as gpool,
            tc.tile_pool(name="blk", bufs=3) as bpool,
            tc.tile_pool(name="psum", bufs=2, space="PSUM") as ppool,
        ):
            w1sb = cpool.tile([P, KC, FEAT], bf16)
            w2sb = cpool.tile([P, KC, FEAT], bf16)
            for k in range(KC):
                nc.sync.dma_start(out=w1sb[:, k, :], in_=w1_h[k * P : (k + 1) * P, :])
                nc.sync.dma_start(out=w2sb[:, k, :], in_=w2_h[k * P : (k + 1) * P, :])
            b1sb = cpool.tile([P, FEAT], f32)
            nc.sync.dma_start(out=b1sb[:], in_=b1_h[:])
            b2sb = cpool.tile([P, FEAT], f32)
            nc.sync.dma_start(out=b2sb[:], in_=b2_h[:])

            def matmul_block(xT_grp, j, w_sb, b_sb, htile, ptag):
                # htile = relu(xT_grp_j.T @ w + bias): PE matmuls accumulate in
                # PSUM, DVE adds the (pre-broadcast) bias on eviction, ACT relus
                # in place.
                for h in range(2):
                    pt = ppool.tile([P, NH], f32, tag=f"{ptag}{h}")
                    for k in range(KC):
                        nc.tensor.matmul(
                            pt[:],
                            lhsT=xT_grp[:, k, j * P : (j + 1) * P],
                            rhs=w_sb[:, k, h * NH : (h + 1) * NH],
                            start=(k == 0),
                            stop=(k == KC - 1),
                        )
                    nc.vector.tensor_tensor(
                        htile[:, h * NH : (h + 1) * NH],
                        pt[:],
                        b_sb[:, h * NH : (h + 1) * NH],
                        add,
                    )
                nc.scalar.activation(htile[:], htile[:], relu)

            # relu(b1) broadcast for the tail rows
            rb1 = cpool.tile([P, FEAT], f32)
            nc.vector.tensor_scalar_max(rb1[:], b1sb[:], 0.0)

            def load_T(dram_h, col0, tag):
                # one DMA per group tile (8 segments of MG*P*2B per partition);
                # Bacc legalizes the multi-queue waits on the consumer side
                t = gpool.tile([P, KC, MG * P], bf16, tag=tag)
                nc.sync.dma_start(
                    out=t[:],
                    in_=dram_h[:, col0 : col0 + MG * P].rearrange(
                        "(kc kp) m -> kp kc m", kp=P
                    ),
                )
                return t

            def load_nat(dram_h, row0, tag):
                # [MG*P, FEAT] natural rows -> [P, MG, FEAT] tile, one DMA
                t = gpool.tile([P, MG, FEAT], bf16, tag=tag)
                nc.sync.dma_start(
                    out=t[:],
                    in_=dram_h[row0 : row0 + MG * P, :].rearrange(
                        "(j p) f -> p j f", p=P
                    ),
                )
                return t

            # ---- tail rows (no messages): out = relu(b1) + relu2 ----
            for g in range(nB_blk // MG):
                col0 = nA + g * MG * P
                tTg = load_T(tT_h, col0, "tTg")
                for j in range(MG):
                    blk = nA_blk + g * MG + j
                    h2 = bpool.tile([P, FEAT], f32, tag="h2")
                    matmul_block(tTg, j, w2sb, b2sb, h2, "p2")
                    outb = bpool.tile([P, FEAT], f32, tag="outb")
                    nc.vector.tensor_tensor(outb[:], h2[:], rb1[:], add)
                    # out-stores issue on gpsimd's SWDGE queue so the sync queue
                    # stays a pure load queue (stores wait on compute and would
                    # head-of-line-block subsequent loads)
                    nc.gpsimd.dma_start(
                        out=out_h[blk * P : (blk + 1) * P, :], in_=outb[:]
                    )

            # ---- message rows: out = 0.5*(o+s) + relu1 + relu2 ----
            for g in range(nA_blk // MG):
                col0 = g * MG * P
                oTg = load_T(oT_h, col0, "oTg")
                sTg = load_T(sT_h, col0, "sTg")
                srT = gpool.tile([P, KC, MG * P], bf16, tag="srT")
                nc.vector.tensor_tensor(srT[:], oTg[:], sTg[:], add)
                tTg = load_T(tT_h, col0, "tTg")
                o_g = load_nat(o_h, col0, "o_g")
                s_g = load_nat(s_h, col0, "s_g")
                # in place: o_g becomes o+s
                nc.vector.tensor_tensor(o_g[:], o_g[:], s_g[:], add)
                for j in range(MG):
                    blk = g * MG + j
                    h1 = bpool.tile([P, FEAT], f32, tag="h1")
                    matmul_block(srT, j, w1sb, b1sb, h1, "p1")
                    h2 = bpool.tile([P, FEAT], f32, tag="h2")
                    matmul_block(tTg, j, w2sb, b2sb, h2, "p2")

                    outb = bpool.tile([P, FEAT], f32, tag="outb")
                    nc.vector.tensor_tensor(outb[:], h1[:], h2[:], add)
                    # outb = (0.5 * (o+s)) + outb
                    nc.vector.scalar_tensor_tensor(
                        outb[:], o_g[:, j, :], 0.5, outb[:], mult, add
                    )
                    # out-stores issue on gpsimd's SWDGE queue so the sync queue
                    # stays a pure load queue (stores wait on compute and would
                    # head-of-line-block subsequent loads)
                    nc.gpsimd.dma_start(
                        out=out_h[blk * P : (blk + 1) * P, :], in_=outb[:]
                    )

    nc.finalize()
    return nc


def _get_program(nA_blk, nB_blk):
    key = (nA_blk, nB_blk)
    if key not in _BUILD_CACHE:
        _BUILD_CACHE[key] = _build(nA_blk, nB_blk)
    return _BUILD_CACHE[key]


def _pad_rows(a, n):
    if a.shape[0] == n:
        return a
    out = np.zeros((n,) + a.shape[1:], dtype=a.dtype)
    out[: a.shape[0]] = a
    return out


def _ceil_blocks(n):
    blks = -(-n // P)
    return -(-blks // MG) * MG  # round up to MG blocks


def kernel(
    msg_from_o,
    msg_from_s,
    o_ava_idx,
    s_ava_idx,
    tar_feat,
    w1_W,
    w1_b,
    w2_W,
    w2_b,
):
    global LAST_RESULT
    from concourse.bass_utils import run_bass_kernel_spmd

    n_tar, feat = tar_feat.shape
    n_msg = msg_from_o.shape[0]
    assert feat == FEAT

    idx_o = np.asarray(o_ava_idx)
    idx_s = np.asarray(s_ava_idx)
    identity_idx = (
        n_msg == len(idx_o)
        and n_msg == len(idx_s)
        and np.array_equal(idx_o, np.arange(n_msg, dtype=idx_o.dtype))
        and np.array_equal(idx_s, np.arange(n_msg, dtype=idx_s.dtype))
    )

    msg_o = np.asarray(msg_from_o, dtype=np.float32)
    msg_s = np.asarray(msg_from_s, dtype=np.float32)
    tar = np.asarray(tar_feat, dtype=np.float32)

    if not identity_idx:
        # general path: scatter on host, treat every row as a message row
        po = np.zeros_like(tar)
        po[idx_o] = msg_o
        ps = np.zeros_like(tar)
        ps[idx_s] = msg_s
        msg_o, msg_s = po, ps
        n_msg_eff = n_tar
    else:
        n_msg_eff = n_msg

    n_tail = n_tar - n_msg_eff
    mA = -(-n_msg_eff // N_CORES)
    mB = -(-n_tail // N_CORES) if n_tail else 0
    nA_pad = _ceil_blocks(mA) * P
    nB_pad = _ceil_blocks(mB) * P if mB > 0 else 0

    nc = _get_program(nA_pad // P, nB_pad // P)

    # replicated weights (bf16 casts are bit-identical to an on-device cast)
    w1t = np.ascontiguousarray((0.5 * np.asarray(w1_W, np.float32).T)).astype(BF16)
    w2t = np.ascontiguousarray(np.asarray(w2_W, np.float32).T).astype(BF16)
    b1bc = np.ascontiguousarray(
        np.broadcast_to(np.asarray(w1_b, np.float32), (P, FEAT))
    )
    b2bc = np.ascontiguousarray(
        np.broadcast_to(np.asarray(w2_b, np.float32), (P, FEAT))
    )

    in_maps = []
    bounds = []
    for c in range(N_CORES):
        a0 = min(c * mA, n_msg_eff)
        a1 = min(a0 + mA, n_msg_eff)
        b0 = min(n_msg_eff + c * mB, n_tar)
        b1 = min(b0 + mB, n_tar)
        bounds.append((a0, a1, b0, b1))

        o_c = _pad_rows(msg_o[a0:a1], nA_pad).astype(BF16)
        s_c = _pad_rows(msg_s[a0:a1], nA_pad).astype(BF16)
        tarA = _pad_rows(tar[a0:a1], nA_pad)
        tarB = _pad_rows(tar[b0:b1], nB_pad)
        tcat = np.concatenate([tarA, tarB], axis=0)

        in_maps.append(
            {
                "o": o_c,
                "s": s_c,
                "oT": np.ascontiguousarray(o_c.T),
                "sT": np.ascontiguousarray(s_c.T),
                "tT": np.ascontiguousarray(tcat.T).astype(BF16),
                "w1t": w1t,
                "w2t": w2t,
                "b1bc": b1bc,
                "b2bc": b2bc,
            }
        )

    br = run_bass_kernel_spmd(
        nc,
        in_maps,
        list(range(N_CORES)),
        trace=TRACE,
        **TRACE_KWARGS,
    )
    LAST_RESULT = br

    out = np.empty((n_tar, FEAT), dtype=np.float32)
    for c, (a0, a1, b0, b1) in enumerate(bounds):
        res = br.results[c]["out"]
        out[a0:a1] = res[: a1 - a0]
        if b1 > b0:
            out[b0:b1] = res[nA_pad : nA_pad + (b1 - b0)]
    return out


# revision 19
# speedup vs baseline: 1.1192x; 1.1192x over previous
"""Trainium2 Bass kernel for nn_FeatureRefiner (gnn_message_passing).

Computes, for full inputs:
    padded_o = zeros[60000,1024].at[o_ava_idx].set(msg_from_o)
    padded_s = zeros[60000,1024].at[s_ava_idx].set(msg_from_s)
    summ   = 0.5*(padded_o + padded_s)
    out    = summ + relu(summ @ w1_W.T + w1_b) + relu(tar_feat @ w2_W.T + w2_b)

Strategy: data-parallel over rows across 8 NeuronCores. Message rows need
both matmuls; rows with no incoming message (summ == 0) only need the w2
matmul plus a relu(w1_b) broadcast, so each core gets 1/8 of the message
rows and 1/8 of the tail rows to balance FLOPs. Matmul operands are
host-pre-transposed bf16 (summ^T is formed on-device as 0.5*(o^T + s^T)
with the 0.5 folded into w1t); the fp32 summ addend uses natural-layout
fp32 message tiles so it stays exact. Biases ride the PE as a ones⊗bias
matmul appended to each accumulation group; ACT applies ReLU on PSUM
eviction.
"""

import sys

sys.path.insert(0, "/opt/trn_rl_repo")

import numpy as np
import ml_dtypes

FEAT = 1024
P = 128
KC = FEAT // P  # 8 contraction chunks
NH = 512  # matmul moving free dim (one PSUM bank of fp32)
MG = 4  # blocks per transposed-operand load group
N_CORES = 8

BF16 = ml_dtypes.bfloat16

TRACE = False
TRACE_KWARGS = {}
LAST_RESULT = None

_BUILD_CACHE = {}


def _build(nA_blk, nB_blk):
    """Build the single-core Tile program (same program runs SPMD on all cores).

    nA_blk: number of 128-row blocks of message rows (both matmuls)
    nB_blk: number of 128-row blocks of tail rows (w2 matmul only)
    """
    import concourse.bass as bass  # noqa: F401
    import concourse.bacc as bacc
    import concourse.mybir as mybir
    from concourse import tile

    f32 = mybir.dt.float32
    bf16 = mybir.dt.bfloat16
    add = mybir.AluOpType.add
    mult = mybir.AluOpType.mult
    relu = mybir.ActivationFunctionType.Relu

    nA = nA_blk * P
    nB = nB_blk * P

    nc = bacc.Bacc("TRN2", target_bir_lowering=False, debug=False)

    o_h = nc.declare_dram_parameter("o", [nA, FEAT], bf16, isOutput=False)
    s_h = nc.declare_dram_parameter("s", [nA, FEAT], bf16, isOutput=False)
    oT_h = nc.declare_dram_parameter("oT", [FEAT, nA], bf16, isOutput=False)
    sT_h = nc.declare_dram_parameter("sT", [FEAT, nA], bf16, isOutput=False)
    tT_h = nc.declare_dram_parameter("tT", [FEAT, nA + nB], bf16, isOutput=False)
    w1_h = nc.declare_dram_parameter("w1t", [FEAT, FEAT], bf16, isOutput=False)
    w2_h = nc.declare_dram_parameter("w2t", [FEAT, FEAT], bf16, isOutput=False)
    b1_h = nc.declare_dram_parameter("b1bc", [P, FEAT], f32, isOutput=False)
    b2_h = nc.declare_dram_parameter("b2bc", [P, FEAT], f32, isOutput=False)
    out_h = nc.declare_dram_parameter("out", [nA + nB, FEAT], f32, isOutput=True)

    with tile.TileContext(nc) as tc:
        with (
            tc.tile_pool(name="const", bufs=1) as cpool,
            tc.tile_pool(name="grp", bufs=2) as gpool,
            tc.tile_pool(name="blk", bufs=3) as bpool,
            tc.tile_pool(name="psum", bufs=2, space="PSUM") as ppool,
        ):
            # load order favors the earliest consumers: the tail-row (B) loop
            # only needs w2/b2, so those issue first and the PE can start
            # within ~10us instead of waiting out every constant load
            w2sb = cpool.tile([P, KC, FEAT], bf16)
            nc.sync.dma_start(
                out=w2sb[:], in_=w2_h[:].rearrange("(kc kp) n -> kp kc n", kp=P)
            )
            b2sb = cpool.tile([P, FEAT], f32)
            nc.sync.dma_start(out=b2sb[:], in_=b2_h[:])
            tTg0 = None
            if nB_blk >= MG:
                tTg0 = gpool.tile([P, KC, MG * P], bf16, tag="tTg")
                nc.sync.dma_start(
                    out=tTg0[:],
                    in_=tT_h[:, nA : nA + MG * P].rearrange(
                        "(kc kp) m -> kp kc m", kp=P
                    ),
                )
            w1sb = cpool.tile([P, KC, FEAT], bf16)
            nc.sync.dma_start(
                out=w1sb[:], in_=w1_h[:].rearrange("(kc kp) n -> kp kc n", kp=P)
            )
            b1sb = cpool.tile([P, FEAT], f32)
            nc.sync.dma_start(out=b1sb[:], in_=b1_h[:])

            def matmul_block(xT_grp, j, w_sb, b_sb, htile, ptag):
                # htile = relu(xT_grp_j.T @ w + bias): PE matmuls accumulate in
                # PSUM, DVE adds the (pre-broadcast) bias on eviction, ACT relus
                # in place.
                for h in range(2):
                    pt = ppool.tile([P, NH], f32, tag=f"{ptag}{h}")
                    for k in range(KC):
                        nc.tensor.matmul(
                            pt[:],
                            lhsT=xT_grp[:, k, j * P : (j + 1) * P],
                            rhs=w_sb[:, k, h * NH : (h + 1) * NH],
                            start=(k == 0),
                            stop=(k == KC - 1),
                        )
                    nc.vector.tensor_tensor(
                        htile[:, h * NH : (h + 1) * NH],
                        pt[:],
                        b_sb[:, h * NH : (h + 1) * NH],
                        add,
                    )
                nc.scalar.activation(htile[:], htile[:], relu)

            # relu(b1) broadcast for the tail rows
            rb1 = cpool.tile([P, FEAT], f32)
            nc.vector.tensor_scalar_max(rb1[:], b1sb[:], 0.0)

            def load_T(dram_h, col0, tag):
                # one DMA per group tile (8 segments of MG*P*2B per partition);
                # Bacc legalizes the multi-queue waits on the consumer side
                t = gpool.tile([P, KC, MG * P], bf16, tag=tag)
                nc.sync.dma_start(
                    out=t[:],
                    in_=dram_h[:, col0 : col0 + MG * P].rearrange(
                        "(kc kp) m -> kp kc m", kp=P
                    ),
                )
                return t

            def load_nat(dram_h, row0, tag):
                # [MG*P, FEAT] natural rows -> [P, MG, FEAT] tile, one DMA
                t = gpool.tile([P, MG, FEAT], bf16, tag=tag)
                nc.sync.dma_start(
                    out=t[:],
                    in_=dram_h[row0 : row0 + MG * P, :].rearrange(
                        "(j p) f -> p j f", p=P
                    ),
                )
                return t

            # ---- tail rows (no messages): out = relu(b1) + relu2 ----
            for g in range(nB_blk // MG):
                col0 = nA + g * MG * P
                tTg = tTg0 if g == 0 else load_T(tT_h, col0, "tTg")
                for j in range(MG):
                    blk = nA_blk + g * MG + j
                    h2 = bpool.tile([P, FEAT], f32, tag="h2")
                    matmul_block(tTg, j, w2sb, b2sb, h2, "p2")
                    outb = bpool.tile([P, FEAT], f32, tag="outb")
                    nc.vector.tensor_tensor(outb[:], h2[:], rb1[:], add)
                    nc.sync.dma_start(
                        out=out_h[blk * P : (blk + 1) * P, :], in_=outb[:]
                    )

            # ---- message rows: out = 0.5*(o+s) + relu1 + relu2 ----
            for g in range(nA_blk // MG):
                col0 = g * MG * P
                oTg = load_T(oT_h, col0, "oTg")
                sTg = load_T(sT_h, col0, "sTg")
                srT = gpool.tile([P, KC, MG * P], bf16, tag="srT")
                nc.vector.tensor_tensor(srT[:], oTg[:], sTg[:], add)
                tTg = load_T(tT_h, col0, "tTg")
                o_g = load_nat(o_h, col0, "o_g")
                s_g = load_nat(s_h, col0, "s_g")
                # in place: o_g becomes o+s
                nc.vector.tensor_tensor(o_g[:], o_g[:], s_g[:], add)
                for j in range(MG):
                    blk = g * MG + j
                    h1 = bpool.tile([P, FEAT], f32, tag="h1")
                    matmul_block(srT, j, w1sb, b1sb, h1, "p1")
                    h2 = bpool.tile([P, FEAT], f32, tag="h2")
                    matmul_block(tTg, j, w2sb, b2sb, h2, "p2")

                    outb = bpool.tile([P, FEAT], f32, tag="outb")
                    nc.vector.tensor_tensor(outb[:], h1[:], h2[:], add)
                    # outb = (0.5 * (o+s)) + outb
                    nc.vector.scalar_tensor_tensor(
                        outb[:], o_g[:, j, :], 0.5, outb[:], mult, add
                    )
                    nc.sync.dma_start(
                        out=out_h[blk * P : (blk + 1) * P, :], in_=outb[:]
                    )

    nc.finalize()
    return nc


def _get_program(nA_blk, nB_blk):
    key = (nA_blk, nB_blk)
    if key not in _BUILD_CACHE:
        _BUILD_CACHE[key] = _build(nA_blk, nB_blk)
    return _BUILD_CACHE[key]


def _pad_rows(a, n):
    if a.shape[0] == n:
        return a
    out = np.zeros((n,) + a.shape[1:], dtype=a.dtype)
    out[: a.shape[0]] = a
    return out


def _ceil_blocks(n):
    blks = -(-n // P)
    return -(-blks // MG) * MG  # round up to MG blocks


def kernel(
    msg_from_o,
    msg_from_s,
    o_ava_idx,
    s_ava_idx,
    tar_feat,
    w1_W,
    w1_b,
    w2_W,
    w2_b,
):
    global LAST_RESULT
    from concourse.bass_utils import run_bass_kernel_spmd

    n_tar, feat = tar_feat.shape
    n_msg = msg_from_o.shape[0]
    assert feat == FEAT

    idx_o = np.asarray(o_ava_idx)
    idx_s = np.asarray(s_ava_idx)
    identity_idx = (
        n_msg == len(idx_o)
        and n_msg == len(idx_s)
        and np.array_equal(idx_o, np.arange(n_msg, dtype=idx_o.dtype))
        and np.array_equal(idx_s, np.arange(n_msg, dtype=idx_s.dtype))
    )

    msg_o = np.asarray(msg_from_o, dtype=np.float32)
    msg_s = np.asarray(msg_from_s, dtype=np.float32)
    tar = np.asarray(tar_feat, dtype=np.float32)

    if not identity_idx:
        # general path: scatter on host, treat every row as a message row
        po = np.zeros_like(tar)
        po[idx_o] = msg_o
        ps = np.zeros_like(tar)
        ps[idx_s] = msg_s
        msg_o, msg_s = po, ps
        n_msg_eff = n_tar
    else:
        n_msg_eff = n_msg

    n_tail = n_tar - n_msg_eff
    mA = -(-n_msg_eff // N_CORES)
    mB = -(-n_tail // N_CORES) if n_tail else 0
    nA_pad = _ceil_blocks(mA) * P
    nB_pad = _ceil_blocks(mB) * P if mB > 0 else 0

    nc = _get_program(nA_pad // P, nB_pad // P)

    # replicated weights (bf16 casts are bit-identical to an on-device cast)
    w1t = np.ascontiguousarray((0.5 * np.asarray(w1_W, np.float32).T)).astype(BF16)
    w2t = np.ascontiguousarray(np.asarray(w2_W, np.float32).T).astype(BF16)
    b1bc = np.ascontiguousarray(
        np.broadcast_to(np.asarray(w1_b, np.float32), (P, FEAT))
    )
    b2bc = np.ascontiguousarray(
        np.broadcast_to(np.asarray(w2_b, np.float32), (P, FEAT))
    )

    in_maps = []
    bounds = []
    for c in range(N_CORES):
        a0 = min(c * mA, n_msg_eff)
        a1 = min(a0 + mA, n_msg_eff)
        b0 = min(n_msg_eff + c * mB, n_tar)
        b1 = min(b0 + mB, n_tar)
        bounds.append((a0, a1, b0, b1))

        o_c = _pad_rows(msg_o[a0:a1], nA_pad).astype(BF16)
        s_c = _pad_rows(msg_s[a0:a1], nA_pad).astype(BF16)
        tarA = _pad_rows(tar[a0:a1], nA_pad)
        tarB = _pad_rows(tar[b0:b1], nB_pad)
        tcat = np.concatenate([tarA, tarB], axis=0)

        in_maps.append(
            {
                "o": o_c,
                "s": s_c,
                "oT": np.ascontiguousarray(o_c.T),
                "sT": np.ascontiguousarray(s_c.T),
                "tT": np.ascontiguousarray(tcat.T).astype(BF16),
                "w1t": w1t,
                "w2t": w2t,
                "b1bc": b1bc,
                "b2bc": b2bc,
            }
        )

    br = run_bass_kernel_spmd(
        nc,
        in_maps,
        list(range(N_CORES)),
        trace=TRACE,
        **TRACE_KWARGS,
    )
    LAST_RESULT = br

    out = np.empty((n_tar, FEAT), dtype=np.float32)
    for c, (a0, a1, b0, b1) in enumerate(bounds):
        res = br.results[c]["out"]
        out[a0:a1] = res[: a1 - a0]
        if b1 > b0:
            out[b0:b1] = res[nA_pad : nA_pad + (b1 - b0)]
    return out
